# revision 37
# baseline (speedup 1.0000x reference)
"""DTW (symmetric2, L1 cost) batch kernel for Trainium2, 8 NeuronCores.

Problem: 64 pairs of length-1024 fp32 sequences; per pair the full
1024x1024 DTW dynamic program; output = mean over pairs of
D[n-1, m-1] / (n + m).

Strategy per core (8 samples each):
  - Row-scan formulation on offset costs d' = d - COFF (exact: every
    DTW path to (i,j) accumulates local costs with multiplicity
    i+j+1, so E = D - COFF*(i+j+1) satisfies the same recurrence):
        q[j] = Eprev[j-1] + d'[j]
        p[j] = min(q[j], Eprev[j])
        E[j] = min(p[j], E[j-1]) + d'[j]
    The serial in-row recurrence maps onto the DVE tensor_tensor_scan
    instruction (op0=min, op1=add).  The offset keeps E small near the
    optimal path so all DP state can be stored fp16, which enables the
    DVE 2x performance mode for the two tensor_tensor ops (94ns vs
    127ns each; the scan has no fast mode).
  - Columns split into 16 chunks of 64; partition p = 8*chunk + sample.
    Chunks run in a software wavefront: chunk c processes row block
    i//R at macro-step tau = i//R + SKEW*c (the skew gives the
    boundary transfer R*(SKEW-1) rows of slack).
  - Row state lives in BB[128, 16, 65]: slot s%16 holds row s; col 0
    is the LEFT-NEIGHBOR boundary (chunk c-1's row-s last column,
    i.e. a +8 partition shift), cols 1..64 are the scan output.
    The scan reads col 0 as its initial carry; q reads cols 0..63.
  - The chunk->chunk boundary move (a +8 partition shift, illegal for
    DVE access patterns) is done by DMA once per macro-step: col W of
    the macro-step's R slots -> col 0 of the slots R*SKEW rows ahead,
    partitions shifted by +8.  R*(SKEW-1) rows separate the trigger
    from the first consumer, hiding the DMA latency; PE/PSUM are not
    used.
  - Local cost rows d'[j] = |x_i - y_j| - COFF are produced by the
    Scalar (activation) engine (Abs into an fp32 scratch) followed by
    GPSIMD (subtract COFF, downcast to fp16), both off the critical
    path.
  - After Tile scheduling, semaphore waits that same-engine program
    order already guarantees are stripped; they otherwise serialize
    the DVE sequencer against its own engine (~2x slowdown).
"""

import sys

sys.path.insert(0, "/opt/trn_rl_repo")

import numpy as np

import concourse.bass as bass
import concourse.bacc as bacc
import concourse.mybir as mybir
from concourse import tile
from concourse.bass_utils import run_bass_kernel_spmd

AF = mybir.ActivationFunctionType
ALU = mybir.AluOpType
FP32 = mybir.dt.float32
FP16 = mybir.dt.float16

NCORES = 8
B = 8             # samples per core
N = 1024          # sequence length (rows == cols)
C = 16            # column chunks
W = N // C        # 64 columns per chunk
R = 4             # rows per macro-step
SKEW = 3          # macro-steps of lag between adjacent chunks
T = N // R + SKEW * (C - 1)   # 158 macro-steps
S_TOTAL = T * R               # 1264 row-steps
NSLOT = 16                    # row-state arena slots
COFF = 0.3        # local-cost offset c: run the DP on d' = d - c.  Every
                  # path to (i,j) accumulates local costs with total
                  # multiplicity i+j+1, so this is exact with
                  # E = D - c*(i+j+1); near the optimal path E stays small,
                  # which makes fp16 storage of the DP state accurate
                  # (measured ~2e-5 rel err vs ~4e-2 for fp16 without it).
BIGW = 5000.0     # fp16-finite "infinity": far above any live DP value
                  # (max live |E| ~ 900) yet small enough that dead-lane
                  # accumulation stays fp16-finite
XSPAD = 200.0     # dead-lane x pad: dead DP values grow ~200/row, ending
                  # <= 5000 + 200*180 = 41000 < fp16 max

_CACHE = {}


def _strip_same_engine_waits(nc):
    """Remove semaphore waits that same-engine program order already
    guarantees.

    Tile emits a sem wait for every data dependency, including ones
    between two instructions on the same (in-order) engine.  Such a wait
    forces the sequencer to stall until the producer's completion count
    propagates back, exposing ~100-200ns per instruction on the serial
    DP chain.  For an in-order engine the data hazard is already
    resolved by queue order, so a wait on the engine's own completion
    counter whose target value is covered by instructions earlier in
    program order can be dropped.
    """
    fn = nc.m.functions[0]
    insts = [i for b in fn.blocks for i in b.instructions]

    # sem id -> set of engines whose instructions update it
    updaters = {}
    for inst in insts:
        si = inst.sync_info
        if si is None:
            continue
        for u in si.on_update:
            if u.sync_type == "semaphore":
                updaters.setdefault(u.id, set()).add(inst.engine)

    stripped = 0
    cum = {}  # sem id -> sum of update_values seen so far in program order
    for inst in insts:
        si = inst.sync_info
        if si is None:
            continue
        # Only the hot per-row op types are stripped.  Notably Memset /
        # TensorCopy waits must stay: hardware-bisected — removing the two
        # such waits here (a memset-memset WAW and the final output copy)
        # deterministically corrupts results, so same-engine program order
        # evidently does not cover them on real hardware.
        eng = inst.engine
        kept = []
        for w in si.on_wait:
            if (
                w.sync_type == "semaphore"
                and w.wait_mode == "sem-ge-imm"
                and updaters.get(w.id) == {eng}
                and w.wait_value <= cum.get(w.id, 0)
                and type(inst).__name__
                in ("InstTensorTensor", "InstTensorScalarPtr", "InstActivation")
            ):
                stripped += 1
                continue
            kept.append(w)
        if len(kept) != len(si.on_wait):
            si.on_wait = kept
        for u in si.on_update:
            if u.sync_type == "semaphore" and u.update_mode == "sem-inc":
                cum[u.id] = cum.get(u.id, 0) + u.update_value
    return stripped


def _build():
    nc = bacc.Bacc("TRN2", target_bir_lowering=False, debug=False)
    x8 = nc.declare_dram_parameter("x8", [B, N], FP32, isOutput=False)
    y8 = nc.declare_dram_parameter("y8", [B, N], FP32, isOutput=False)
    out = nc.declare_dram_parameter("dists", [B, 1], FP32, isOutput=True)

    with tile.TileContext(nc) as tc:
        with (
            tc.tile_pool(name="persist", bufs=1) as pp,
            tc.tile_pool(name="qpool", bufs=2) as qpool,
        ):
            Y = pp.tile([128, W], FP32, tag="y")
            XS = pp.tile([128, S_TOTAL], FP32, tag="xs")
            BB = pp.tile([128, NSLOT, W + 1], FP16, tag="bb")
            ZC = pp.tile([128, 1], FP32, tag="zc")
            OUTT = pp.tile([128, 1], FP32, tag="outt")

            # Dummy activation: forces the framework's activation-table
            # load (~1.3us) to run immediately, before the input DMAs land,
            # instead of serializing in front of the first real activation.
            # OUTT is scratch here; it is rewritten at the end.  The
            # gpsimd memset initializes it off the DVE/Act queues.
            nc.gpsimd.memset(OUTT[:], 0.0)
            nc.scalar.activation(OUTT[:], OUTT[:], AF.Abs, bias=0.0, scale=1.0)
            # X skew: XS[8c+b, s] = x[b, s - SKEW*R*c]; pad XSPAD so
            # out-of-range rows produce large (fp16-finite) local costs.
            # The activation computes |Y*(-1) + x| = |x - y|, so XS holds
            # +x directly (no negation pass; early rows start as soon as
            # their own chunk's load lands).
            # Pad only the columns some chunk's load leaves uncovered:
            # chunk c loads cols [SKEW*R*c, SKEW*R*c + N), so the union of
            # pad regions is [0, SKEW*R*(C-1)) and [N, S_TOTAL).
            nc.vector.memset(XS[:, 0 : SKEW * R * (C - 1)], XSPAD)
            nc.vector.memset(XS[:, N:S_TOTAL], XSPAD)
            # Y loads first (the first activation reads all 128 partitions
            # of Y, so every Y load gates row 0), then XS in chunk order
            # (row s only needs the XS loads whose column range covers s,
            # so later chunks' loads land behind the compute).  The DMA
            # trigger ring processes one dma_start per ~625ns.
            # One batched Y load: partition 8c+b gets y[b, 64c:64c+64], so
            # the DRAM side iterates (chunk, sample, col) = strides
            # (W, N, 1) — matching the SBUF side's partition-major order.
            # Row 0's activation reads all of Y, so batching its load into
            # a single DGE trigger (vs 16 x ~625ns) pulls the start in.
            y_src = bass.AP(tensor=y8, offset=0, ap=[[W, C], [N, B], [1, W]])
            nc.sync.dma_start(Y[:], y_src)
            # Only chunk 0's XS load happens up front; chunks 1..15 are
            # issued from inside the macro loop (below), staggered ahead
            # of first use at row SKEW*R*c.  All triggers ride the SP
            # queue: a waiting DMA trigger blocks its in-order queue, so
            # keeping the Act/GPSIMD queues trigger-free lets the
            # activation-table load and the first d' ops start
            # immediately.
            nc.sync.dma_start(XS[0:8, 0:N], x8[:])

            nc.vector.memset(BB[:], BIGW)
            nc.vector.memset(ZC[:], BIGW)
            nc.vector.memset(ZC[0:8, :], 0.0)

            # d' tiles [128, W] fp16 hold |y - x_i| - COFF for DP cols
            # 1..64; produced per row by ACT (|y-x|, fp32 scratch) then
            # GPSIMD (subtract COFF, downcast).  p tiles [128, W] fp16.
            dts = [
                pp.tile([128, W], FP16, name=f"dt{i}", tag=f"dt{i}")
                for i in range(4)
            ]
            dtmp = [
                pp.tile([128, W], FP32, name=f"dm{i}", tag=f"dm{i}")
                for i in range(3)
            ]
            pts = [
                pp.tile([128, W], FP16, name=f"pt{i}", tag=f"pt{i}")
                for i in range(3)
            ]

            for tau in range(T):
                k0 = (R * tau) % NSLOT
                do_dma = tau < T - SKEW
                if tau % 2 == 0 and 1 <= tau // 2 < C:
                    # stagger chunk c's XS load ~R*SKEW*c - 4*c rows ahead
                    # of its first read, without queueing all 15 triggers
                    # ahead of the early boundary DMAs on SP
                    c = tau // 2
                    o = SKEW * R * c
                    nc.sync.dma_start(
                        XS[8 * c : 8 * c + 8, o : o + N], x8[:]
                    )
                for r in range(R):
                    s = R * tau + r
                    b_prev = BB[:, (s - 1) % NSLOT, :]
                    b_cur = BB[:, s % NSLOT, :]
                    dm = dtmp[s % 3]
                    d = dts[s % 4]
                    nc.scalar.activation(
                        dm[:],
                        Y[:],
                        AF.Abs,
                        bias=XS[:, s : s + 1],
                        scale=-1.0,
                    )
                    nc.gpsimd.tensor_scalar_add(d[:], dm[:], -COFF)
                    q = qpool.tile([128, W], FP16, tag="q", name="q")
                    nc.vector.tensor_tensor(
                        q[:], b_prev[:, 0:W], d[:], op=ALU.add
                    )
                    p = pts[s % 3]
                    nc.vector.tensor_tensor(
                        p[:], q[:], b_prev[:, 1 : W + 1], op=ALU.min
                    )
                    init = ZC[:, 0:1] if s == 0 else b_cur[:, 0:1]
                    nc.vector.tensor_tensor_scan(
                        b_cur[:, 1 : W + 1],
                        p[:],
                        d[:],
                        init,
                        op0=ALU.min,
                        op1=ALU.add,
                    )
                if do_dma:
                    # boundary shift: col W of this macro-step's slots ->
                    # col 0 of the slots R*SKEW rows ahead, partitions +8
                    # (chunk c -> chunk c+1).  The consumer rows are
                    # R*(SKEW-1) rows (~2.5us) past this trigger, hiding the
                    # ~1.8us DMA-completion latency; the destination slots'
                    # old col-0 values were consumed a full macro-step ago.
                    m = (k0 + R * SKEW) % NSLOT
                    nc.sync.dma_start(
                        BB[8:128, m : m + R, 0:1],
                        BB[0:120, k0 : k0 + R, W : W + 1],
                    )

            last_cur = BB[:, (S_TOTAL - 1) % NSLOT, :]
            nc.vector.tensor_copy(OUTT[:], last_cur[:, W : W + 1])
            nc.sync.dma_start(out[:], OUTT[120:128, :])

    import os

    if not os.environ.get("DTW_NOSTRIP"):
        _strip_same_engine_waits(nc)
    nc.compile()
    return nc


def _in_maps(x, y):
    return [
        {"x8": x[8 * k : 8 * k + 8], "y8": y[8 * k : 8 * k + 8]}
        for k in range(NCORES)
    ]


def kernel(x: np.ndarray, x_target: np.ndarray) -> np.ndarray:
    x = np.ascontiguousarray(np.asarray(x, np.float32))
    y = np.ascontiguousarray(np.asarray(x_target, np.float32))
    if "nc" not in _CACHE:
        _CACHE["nc"] = _build()
    nc = _CACHE["nc"]
    in_maps = _in_maps(x, y)
    res = run_bass_kernel_spmd(nc, in_maps, list(range(NCORES))).results
    dists = np.concatenate([r["dists"][:, 0] for r in res]).astype(np.float64)
    # E[n-1,n-1] = D[n-1,n-1] - COFF*(2N-1); normalize by n + m = 2N
    dists = (dists + COFF * (2 * N - 1)) / np.float64(2.0 * N)
    return np.float32(np.mean(dists))
